# revision 1
# baseline (speedup 1.0000x reference)
"""Trainium2 Bass kernel for the pairwise-MLP geometric convolution.

Reference computes, per batch z:
    rel[a,b]   = g[b] - g[a]
    h[a,b,:]   = relu(rel @ W1 + b1)                      [N,N,H]
    k[a,b,:]   = h @ W2 + b2  -> [N,N,C_OUT,C_IN]
    out[a,i]   = sum_{b,j} k[a,b,i,j] * f[b,j]

Key factorization (avoids materializing k, 537MB -> ~1MB):
    U = g @ W1  (so rel@W1 = U[b]-U[a])
    G[b,h,i]   = sum_j W2[h, i*C_IN+j] * f[b,j]
    out[a,i]   = sum_{b,h} relu(U[b,h]+b1[h]-U[a,h]) * G[b,h,i]
               + sum_j b2[i,j] * (sum_b f[b,j])

Sharding over 8 cores: z (2) x b-quarter (4). Each core computes the full
[i=32, a=256] transposed partial for its 64 b's; host sums quarters and
transposes. Contraction runs on the PE as 32 accumulating matmuls with
K=128 chunks of (b-pair x 64 h): stationary G chunk [128,32], moving
T chunk [128,256] built by one fused tensor_scalar (add bias, relu) per
chunk, spread across DVE/ACT/GPSIMD.

Hardware constraint honored throughout: a PE Matmult can carry at most ONE
sync-wait, so all small inputs arrive in a single packed DMA, and two
dummy matmuls make the PE observe the two g_sb gather DMAs before the
main accumulation chain.
"""

import os
import sys

import numpy as np

_TRN_REPO = "/opt/trn_rl_repo"
if _TRN_REPO not in sys.path:
    sys.path.insert(0, _TRN_REPO)

from contextlib import ExitStack

import concourse.bass as bass
import concourse.mybir as mybir
import concourse.tile as tile
from concourse.bass_utils import run_bass_kernel_spmd

from concourse.vector_clock import ScopedClock

# The walrus codegen used on the axon/PJRT path accepts at most ONE sync-wait
# per TPB instruction. Tile's kernel-tail drain aggregates a wait for every
# live semaphore onto a single Drain, which walrus rejects. Patch the tail to
# spread those waits across single-wait SP nops before an unadorned drain.
_orig_drain_and_barrier = tile.TileContext._drain_and_barrier


def _split_wait_drain_and_barrier(self, tick_clock, wait_clock):
    nc = self.nc
    probe = nc.sync.nop(nofuse=True)
    wait_clock.add_sem_waits(probe.ins, ScopedClock({None: tick_clock.global_clock}))
    si = probe.ins.sync_info
    waits = list(si.on_wait) if si is not None and si.on_wait else []
    if len(waits) > 1:
        probe.ins.sync_info = mybir.SyncInfo(on_wait=waits[:1], on_update=[])
        for w in waits[1:]:
            extra = nc.sync.nop(nofuse=True)
            extra.ins.sync_info = mybir.SyncInfo(on_wait=[w], on_update=[])
    nc.sync.drain()
    nc.all_engine_barrier()
    popped = nc._tile_sem_poison_stack.pop()
    assert popped is self._sem_poison
    nc.clear_and_free_semaphores(list(self.sems.allocated().values()))
    nc.all_engine_barrier()


tile.TileContext._drain_and_barrier = _split_wait_drain_and_barrier

F32 = mybir.dt.float32
# bf16 runs the PE at 1 cycle/row vs 4 for fp32; accumulation stays fp32 in
# PSUM. Only the big contraction operands (T, G) are bf16.
BF16 = mybir.dt.bfloat16
Z, N, C_IN, C_OUT, H = 2, 256, 32, 32, 64
BQ = 64          # b-points per core (N / 4 quarters)
NPAIR = BQ // 2  # 32 K-chunks of (2 b x 64 h) = 128

# packed small-input tensor layout (fp32): [64, PKW]
#   cols 0:256    fTfull   (parts 0:32)
#   cols 256:288  b2T      (parts 0:32)
#   col  288      b1c      (parts 0:64)
PKW = 289
# bf16 packed tensor (matmul operands), loaded as two DMAs (cols 0:MA,
# MA:MPW) so the U and first G' matmuls start before the whole tensor lands:
#   cols 0:64       fTq      (parts 0:32)
#   cols 64:320     gT       (parts 0:3)
#   cols 320:384    gTb      (parts 0:3)
#   cols 384:448    W1       (parts 0:3)
#   cols 448:2496   M2p
MPW = 2496
MA = 1472

# engine for each of the 32 T-chunk builds: v=vector(DVE), s=scalar(ACT),
# g=gpsimd. ACT carries the shared prep, DVE the G copies.
T_ENGINES = ["g", "s", "v", "g", "s", "g", "s", "v"] * 4
# PE warm-up matmuls between the G' matmuls and the main chain.
N_WARMERS = 16


def build_nc(debug: bool = False) -> bass.Bass:
    nc = bass.Bass("TRN2", target_bir_lowering=False, debug=debug, num_devices=8)

    m2p = nc.dram_tensor("M2p", [C_IN, MPW], BF16, kind="ExternalInput").ap()
    pk = nc.dram_tensor("pk", [H, PKW], F32, kind="ExternalInput").ap()
    outp = nc.dram_tensor("outp", [C_OUT, N], F32, kind="ExternalOutput").ap()

    with tile.TileContext(nc) as tc, ExitStack() as ctx:
        consts = ctx.enter_context(tc.tile_pool(name="consts", bufs=1))
        work = ctx.enter_context(tc.tile_pool(name="work", bufs=1))
        # bufs=NPAIR: every T tile gets its own slot, so no T-op ever waits
        # for a PE slot release (keeps every instruction at <=1 sync wait,
        # a walrus codegen hard limit).
        tpool = ctx.enter_context(tc.tile_pool(name="tpool", bufs=NPAIR))
        psum = ctx.enter_context(tc.tile_pool(name="psum", bufs=1, space="PSUM"))
        dpool = ctx.enter_context(tc.tile_pool(name="dpool", bufs=1, space="DRAM"))

        # ---- input loads. pk goes through the Pool SWDGE queue so the SP
        # HWDGE ring stays within 8 DMAs (no semaphore-lane reuse).
        m2p_sb = consts.tile([C_IN, MPW], BF16)
        nc.sync.dma_start(out=m2p_sb[:, 0:MA], in_=m2p[:, 0:MA])
        nc.sync.dma_start(out=m2p_sb[:, MA:MPW], in_=m2p[:, MA:MPW])
        pk_sb = consts.tile([H, PKW], F32)
        nc.gpsimd.dma_start(out=pk_sb, in_=pk)

        fTq_bf = m2p_sb[:, 0:64]
        gT_bf = m2p_sb[0:3, 64:320]
        gTb_bf = m2p_sb[0:3, 320:384]
        w1_bf = m2p_sb[0:3, 384:448]
        fTfull_sb = pk_sb[0:C_IN, 0:256]
        b2t_sb = pk_sb[0:C_IN, 256:288]
        b1_sb = pk_sb[0:H, 288:289]

        # First DVE / ACT ops must observe only the pk DMA semaphore.
        scol = work.tile([C_IN, 1], F32)
        nc.vector.tensor_reduce(out=scol, in_=fTfull_sb,
                                axis=mybir.AxisListType.X, op=mybir.AluOpType.add)
        s_bcast = work.tile([C_IN, N], BF16)
        nc.vector.tensor_scalar(out=s_bcast, in0=scol.broadcast_to([C_IN, N]),
                                scalar1=0.0, scalar2=None,
                                op0=mybir.AluOpType.add)
        b2t_bf = work.tile([C_IN, C_OUT], BF16)
        nc.vector.tensor_copy(b2t_bf, b2t_sb)

        # ---- U matmuls: U^T = W1^T @ g^T (bf16 in, fp32 accumulate).
        # Both U results share one PSUM bank, freeing a bank for the
        # warm-up matmuls.
        u_ps = psum.tile([H, N + BQ], F32)
        uaT_ps = u_ps[:, 0:N]
        ubT_ps = u_ps[:, N:N + BQ]
        nc.tensor.matmul(uaT_ps, lhsT=w1_bf, rhs=gT_bf, start=True, stop=True)
        nc.tensor.matmul(ubT_ps, lhsT=w1_bf, rhs=gTb_bf, start=True, stop=True)

        # All shared T-op inputs are produced on ACT so T consumers on any
        # engine need exactly one (ACT) wait. negUa duplicated on both
        # partition halves: [128, N].
        negua2 = work.tile([2 * H, N], F32)
        nc.scalar.activation(negua2[0:H, :], uaT_ps,
                             mybir.ActivationFunctionType.Copy, scale=-1.0)
        nc.scalar.activation(negua2[H:2 * H, :], uaT_ps,
                             mybir.ActivationFunctionType.Copy, scale=-1.0)

        # Ub + b1, then stacked by pair: ubT2[bl*H+h, p] = Ub[2p+bl, h] + b1[h]
        ubB = work.tile([H, BQ], F32)
        nc.vector.tensor_scalar(out=ubB, in0=ubT_ps, scalar1=b1_sb,
                                scalar2=None, op0=mybir.AluOpType.add)
        ubT2 = work.tile([2 * H, NPAIR], F32)
        ubB_r = ubB.rearrange("h (p two) -> h two p", two=2)
        nc.scalar.activation(ubT2[0:H, :], ubB_r[:, 0, :],
                             mybir.ActivationFunctionType.Copy)
        nc.scalar.activation(ubT2[H:2 * H, :], ubB_r[:, 1, :],
                             mybir.ActivationFunctionType.Copy)

        # ---- G: G'[b, h*32+i] = sum_j fTq[j,b] * M2p[j, h*32+i] ----
        g_ps = []
        for k in range(4):
            gp = psum.tile([BQ, 512], F32, name=f"g_ps{k}", tag=f"g_ps{k}")
            nc.tensor.matmul(gp, lhsT=fTq_bf,
                             rhs=m2p_sb[:, 448 + k * 512:448 + (k + 1) * 512],
                             start=True, stop=True)
            g_ps.append(gp)

        # PSUM -> SBUF on DVE (DMA cannot read PSUM), then bounce through
        # DRAM to regroup (b-pair, h) onto partitions.
        g_tmp = work.tile([BQ, H * C_OUT], BF16)
        for k in range(4):
            nc.vector.tensor_copy(g_tmp[:, k * 512:(k + 1) * 512], g_ps[k])
        g_sb = work.tile([2 * H, NPAIR, C_OUT], BF16)
        g_dram = dpool.tile([BQ, H * C_OUT], BF16)
        nc.sync.dma_start(out=g_dram, in_=g_tmp)
        # Two gathers split by p-half. Because 64 h * 32 i = 2048 = the
        # g_dram row stride, the (bl, h) pair merges into ONE uniform
        # stride-32 dim, keeping each side a legal 3D AP:
        #   src element (2p+bl, h*32+i) -> offset (bl*64+h)*32 + p*4096 + i
        g0 = g_dram[:, :]
        for ph in range(2):
            g_src = bass.AP(tensor=g0.tensor,
                            offset=g0.offset + ph * 16 * 4096,
                            ap=[[32, 2 * H], [4096, 16], [1, C_OUT]])
            nc.sync.dma_start(out=g_sb[:, 16 * ph:16 * (ph + 1), :],
                              in_=g_src)

        # ---- b2 bias term first in the acc group ----
        acc = psum.tile([C_OUT, N], F32)
        nc.tensor.matmul(acc, lhsT=b2t_bf, rhs=s_bcast, start=True, stop=False)

        scrap = psum.tile([C_OUT, 1], F32)

        def observe_gather(ph):
            # PE observes the p-half gather (one wait) so the following
            # main matmuls need only their T-tile wait.
            nc.tensor.matmul(scrap, lhsT=g_sb[:, 16 * ph, :],
                             rhs=g_sb[:, 16 * ph, 0:1],
                             start=True, stop=True)

        # ---- main contraction: acc[i, a] += G_p^T @ T_p ----
        # T-gated PE warm-up: warmer w consumes t_w as it is produced, so
        # the PE tracks T production (staying at high p-state) instead of
        # idling while the G gathers are in flight.
        warm_ps = psum.tile([C_OUT, N], F32)
        t_tiles = []
        for p in range(NPAIR):
            t_p = tpool.tile([2 * H, N], BF16, tag="T", name=f"t_{p}")
            t_tiles.append(t_p)
            eng = T_ENGINES[p]
            if eng == "s":
                nc.scalar.activation(t_p, negua2,
                                     mybir.ActivationFunctionType.Relu,
                                     bias=ubT2[:, p:p + 1], scale=1.0)
            else:
                e = nc.vector if eng == "v" else nc.gpsimd
                e.tensor_scalar(out=t_p, in0=negua2,
                                scalar1=ubT2[:, p:p + 1], scalar2=0.0,
                                op0=mybir.AluOpType.add,
                                op1=mybir.AluOpType.max)
            if p < N_WARMERS:
                nc.tensor.matmul(warm_ps, lhsT=t_p[0:C_IN, 0:C_OUT],
                                 rhs=t_p[0:C_IN, :], start=True, stop=True)
        for ph in range(2):
            observe_gather(ph)
            for p in range(16 * ph, 16 * (ph + 1)):
                nc.tensor.matmul(acc, lhsT=g_sb[:, p, :], rhs=t_tiles[p],
                                 start=False, stop=(p == NPAIR - 1))

        # ---- store ----
        out_sb = work.tile([C_OUT, N], F32)
        nc.scalar.activation(out_sb, acc, mybir.ActivationFunctionType.Copy)
        nc.sync.dma_start(out=outp, in_=out_sb)

    return nc


def shard_inputs(features, geometry, W1, b1, W2, b2) -> list[dict]:
    import ml_dtypes
    bf16 = ml_dtypes.bfloat16
    f = np.ascontiguousarray(np.asarray(features, np.float32))
    g = np.ascontiguousarray(np.asarray(geometry, np.float32))
    W1 = np.ascontiguousarray(np.asarray(W1, np.float32))
    b1 = np.ascontiguousarray(np.asarray(b1, np.float32))
    W2 = np.ascontiguousarray(np.asarray(W2, np.float32))
    b2 = np.ascontiguousarray(np.asarray(b2, np.float32))

    m2p = W2.reshape(H, C_OUT, C_IN).transpose(2, 0, 1).reshape(C_IN, H * C_OUT)
    b2t = np.ascontiguousarray(b2.reshape(C_OUT, C_IN).T)

    maps = []
    for core in range(8):
        z, q = divmod(core, 4)
        sl = slice(q * BQ, (q + 1) * BQ)
        pk = np.zeros((H, PKW), np.float32)
        pk[0:C_IN, 0:256] = f[z].T
        if q == 0:
            pk[0:C_IN, 256:288] = b2t
        pk[0:H, 288] = b1
        mp = np.zeros((C_IN, MPW), bf16)
        mp[:, 0:64] = f[z, sl].T.astype(bf16)
        mp[0:3, 64:320] = g[z].T.astype(bf16)
        mp[0:3, 320:384] = g[z, sl].T.astype(bf16)
        mp[0:3, 384:448] = W1.astype(bf16)
        mp[:, 448:2496] = m2p.astype(bf16)
        maps.append({"pk": pk, "M2p": mp})
    return maps


def unshard(parts: list[np.ndarray]) -> np.ndarray:
    out = np.empty((Z, N, C_OUT), np.float32)
    for z in range(Z):
        acc = parts[4 * z].astype(np.float32)
        for q in range(1, 4):
            acc = acc + parts[4 * z + q]
        out[z] = acc.T
    return out


def kernel(**inputs) -> np.ndarray:
    nc = build_nc(debug=False)
    in_maps = shard_inputs(**inputs)
    res = run_bass_kernel_spmd(nc, in_maps, list(range(8)))
    return unshard([r["outp"] for r in res.results])



# revision 10
# speedup vs baseline: 1.5469x; 1.5469x over previous
"""Trainium2 Bass kernel for the pairwise-MLP geometric convolution.

Reference computes, per batch z:
    rel[a,b]   = g[b] - g[a]
    h[a,b,:]   = relu(rel @ W1 + b1)                      [N,N,H]
    k[a,b,:]   = h @ W2 + b2  -> [N,N,C_OUT,C_IN]
    out[a,i]   = sum_{b,j} k[a,b,i,j] * f[b,j]

Key factorization (avoids materializing k):
    Ua  = g @ W1                (rel@W1 + b1 = Ub' - Ua, b1 folded into Ub')
    Ub' = [g_q, 1] @ [W1; b1]
    G[b,h,i] = sum_j W2[h, i*C_IN+j] * f[b,j]
    out[a,i] = sum_{b,h} relu(Ub'[b,h] - Ua[a,h]) * G[b,h,i]
             + sum_j b2[i,j] * (sum_b f[b,j])

Sharding over 8 cores: z (2) x b-quarter (4). Each core computes the full
[a=256, i=32] partial for its 64 b's; host sums quarters.

Layout trick that avoids any DRAM-bounce regroup of G: G is computed
TRANSPOSED, one matmul per output channel i, with the W2 slice as the
stationary operand:
    g_ps[h, i*64+b] = sum_j m2p2[j, i*64+h] * fTq[j, b]
so h lands on partitions directly. Strided partition-window copies then build
g3[(hl,h), p, i] = G[2p+hl, h, i]  (hl = b parity) in SBUF, and the main
contraction runs as 64 accumulating matmuls with K = (hl,h) = 128:
    acc[a_half, i] += t_p[:, a_half]^T @ g3[:, p, :]
where t_p[(hl,h), a] = relu(Ub'[2p+hl,h] - Ua[a,h]) is one tensor_scalar per
b-pair (bf16 in/out -> 4x DVE mode). The b2 bias is accumulated into its own
PSUM tile off the critical path and added during the output copy.

Hardware constraint honored throughout: a TPB instruction can carry at most
ONE sync-wait (walrus codegen limit). Shared T inputs are placed so every
consumer needs one wait (ub2 on DVE, negua2 halves on DVE+ACT, a Pool fence
op), and dummy PE matmuls observe the g3 copy semaphores before the main
accumulation chain.
"""

import os
import sys

import numpy as np

_TRN_REPO = "/opt/trn_rl_repo"
if _TRN_REPO not in sys.path:
    sys.path.insert(0, _TRN_REPO)

from contextlib import ExitStack

import concourse.bass as bass
import concourse.mybir as mybir
import concourse.tile as tile
from concourse.bass_utils import run_bass_kernel_spmd

from concourse.vector_clock import ScopedClock

# The walrus codegen used on the axon/PJRT path accepts at most ONE sync-wait
# per TPB instruction. Tile's kernel-tail drain aggregates a wait for every
# live semaphore onto a single Drain, which walrus rejects. Patch the tail to
# spread those waits across single-wait SP nops before an unadorned drain.
_orig_drain_and_barrier = tile.TileContext._drain_and_barrier


def _split_wait_drain_and_barrier(self, tick_clock, wait_clock):
    nc = self.nc
    probe = nc.sync.nop(nofuse=True)
    wait_clock.add_sem_waits(probe.ins, ScopedClock({None: tick_clock.global_clock}))
    si = probe.ins.sync_info
    waits = list(si.on_wait) if si is not None and si.on_wait else []
    if len(waits) > 1:
        probe.ins.sync_info = mybir.SyncInfo(on_wait=waits[:1], on_update=[])
        for w in waits[1:]:
            extra = nc.sync.nop(nofuse=True)
            extra.ins.sync_info = mybir.SyncInfo(on_wait=[w], on_update=[])
    nc.sync.drain()
    nc.all_engine_barrier()
    popped = nc._tile_sem_poison_stack.pop()
    assert popped is self._sem_poison
    nc.clear_and_free_semaphores(list(self.sems.allocated().values()))
    nc.all_engine_barrier()


tile.TileContext._drain_and_barrier = _split_wait_drain_and_barrier

F32 = mybir.dt.float32
BF16 = mybir.dt.bfloat16
Z, N, C_IN, C_OUT, H = 2, 256, 32, 32, 64
BQ = 64          # b-points per core (N / 4 quarters)
NPAIR = BQ // 2  # 32 K-chunks of (2 b x 64 h) = 128

# packed bf16 tensor (matmul operands) [32, MPW]:
#   cols 0:256      gT      (parts 0:3)   g[z].T for Ua
#   cols 256:320    gTb1    (parts 0:4)   [g[z,quarter].T; ones] for Ub'
#   cols 320:384    W1      (parts 0:3)
#   cols 384:448    W1b     (parts 0:4)   [W1; b1]
#   cols 448:576    ones    (part 0)      lhsT for the bias rank-1 matmuls
#   cols 576:640    fTq     (parts 0:32)  f[z,quarter].T
#   cols 640:2688   m2p2    (parts 0:32)  m2p2[j, i*64+h] = W2[h, i*C_IN+j]
#   cols 2688:2720  b2t     (parts 0:32)  b2[i*C_IN+j] transposed (q0 only)
MPW = 2720
D1A = 640  # first DMA: everything the U matmuls + bias pipeline needs

# engine for each of the 32 T-chunk builds: v=vector(DVE), g=gpsimd.
# ACT is saturated by the 8 g3 doubling copies.
T_ENGINES = [
    "v", "v", "v", "g", "v", "v", "v", "v",
    "g", "v", "v", "v", "v", "g", "v", "v",
    "v", "v", "g", "v", "v", "v", "v", "g",
    "v", "v", "v", "v", "g", "v", "v", "v",
]


def build_nc(debug: bool = False) -> bass.Bass:
    nc = bass.Bass("TRN2", target_bir_lowering=False, debug=debug, num_devices=8)

    mp = nc.dram_tensor("mp", [C_IN, MPW], BF16, kind="ExternalInput").ap()
    outp = nc.dram_tensor("outp", [N, C_OUT], F32, kind="ExternalOutput").ap()

    with tile.TileContext(nc) as tc, ExitStack() as ctx:
        consts = ctx.enter_context(tc.tile_pool(name="consts", bufs=1))
        work = ctx.enter_context(tc.tile_pool(name="work", bufs=1))
        # bufs=NPAIR: every T tile gets its own slot, so no T-op ever waits
        # for a PE slot release (keeps every instruction at <=1 sync wait).
        tpool = ctx.enter_context(tc.tile_pool(name="tpool", bufs=NPAIR))
        psum = ctx.enter_context(tc.tile_pool(name="psum", bufs=1, space="PSUM"))

        # ---- input loads: two SP HWDGE DMAs (U/bias inputs first, the big
        # m2p2 block second).
        mp_sb = consts.tile([C_IN, MPW], BF16)
        nc.sync.dma_start(out=mp_sb[:, 0:D1A], in_=mp[:, 0:D1A])
        nc.sync.dma_start(out=mp_sb[:, D1A:MPW], in_=mp[:, D1A:MPW])

        gT = mp_sb[0:3, 0:256]
        gTb1 = mp_sb[0:4, 256:320]
        w1 = mp_sb[0:3, 320:384]
        w1b = mp_sb[0:4, 384:448]
        ones_r = mp_sb[0:1, 448:576]
        fTq = mp_sb[:, 576:640]
        b2t_bf = mp_sb[:, 2688:2720]

        # ---- U matmuls: UaT = W1^T @ gT, UbT' = W1b^T @ [gq;1]  (bf16 in,
        # fp32 accumulate). One PSUM tile for both.
        u_ps = psum.tile([H, N + BQ], F32)
        uaT_ps = u_ps[:, 0:N]
        ubT_ps = u_ps[:, N:N + BQ]
        nc.tensor.matmul(uaT_ps, lhsT=w1, rhs=gT, start=True, stop=True)
        nc.tensor.matmul(ubT_ps, lhsT=w1b, rhs=gTb1, start=True, stop=True)

        # ALL shared T-op inputs live on DVE: the tile scheduler emits one
        # watermark wait per distinct producer engine, so a consumer on any
        # engine then needs exactly one (DVE) wait.
        # ub2[(hl,h), p] = Ub'[2p+hl, h]: strided column gather from ubT_ps.
        ub2 = work.tile([2 * H, NPAIR], F32)
        ubT_r = ubT_ps.rearrange("h (p two) -> h two p", two=2)
        nc.vector.tensor_copy(ub2[0:H, :], ubT_r[:, 0, :])
        nc.vector.tensor_copy(ub2[H:2 * H, :], ubT_r[:, 1, :])
        # negua2[(hl,h), a] = -Ua[a, h] on both partition halves (bf16 so the
        # DVE T-ops hit the 4x perf mode).
        negua2 = work.tile([2 * H, N], BF16)
        for hl in range(2):
            nc.vector.tensor_scalar(out=negua2[hl * H:(hl + 1) * H, :],
                                    in0=uaT_ps, scalar1=-1.0, scalar2=None,
                                    op0=mybir.AluOpType.mult)

        # ---- G matmuls: g_ps[h, i*64+b] = sum_j m2p2[j, i*64+h] fTq[j, b].
        # One matmul per i; the W2 slice is the stationary operand so h lands
        # on partitions. 4 PSUM banks, 8 i-slices each.
        gb = []
        for k in range(4):
            gp = psum.tile([BQ, 512], F32, name=f"g_ps{k}", tag=f"g_ps{k}")
            gb.append(gp)
        for i in range(C_OUT):
            k, off = divmod(i, 8)
            nc.tensor.matmul(gb[k][:, off * 64:(off + 1) * 64],
                             lhsT=mp_sb[:, 640 + i * 64:640 + (i + 1) * 64],
                             rhs=fTq, start=True, stop=True)

        # ---- T tiles: t_p[(hl,h), a] = relu(Ub'[2p+hl,h] - Ua[a,h]).
        t_tiles = []
        for p in range(NPAIR):
            t_p = tpool.tile([2 * H, N], BF16, tag="T", name=f"t_{p}")
            t_tiles.append(t_p)
            e = nc.vector if T_ENGINES[p] == "v" else nc.gpsimd
            e.tensor_scalar(out=t_p, in0=negua2,
                            scalar1=ub2[:, p:p + 1], scalar2=0.0,
                            op0=mybir.AluOpType.add,
                            op1=mybir.AluOpType.max)

        # ---- partition-doubling copies on ACT: g3[(hl,h), p, i] =
        # G[2p+hl, h, i] = g_ps[h, i*64 + 2p+hl]. One strided copy per
        # (bank, hl) so each op carries a single PE-semaphore wait and starts
        # as soon as its bank's G matmuls are done.
        g3 = work.tile([2 * H, NPAIR, C_OUT], BF16)
        for k in range(4):
            src = gb[k].rearrange("h (i p two) -> h two p i", two=2, p=NPAIR)
            for hl in range(2):
                nc.scalar.activation(
                    g3[hl * H:(hl + 1) * H, :, 8 * k:8 * (k + 1)],
                    src[:, hl, :, :],
                    mybir.ActivationFunctionType.Copy)

        # ---- bias, off the critical path. Each core reduces only its own
        # b-quarter; the host-side unshard sum over quarters completes
        # sum_b f[z,b,j]. scol sits after the T ops in the DVE queue; all its
        # PE consumers run after the mains.
        scol = work.tile([C_IN, 1], BF16)
        with nc.allow_low_precision(reason="bias rank-1 term, tolerance 2e-2"):
            nc.vector.tensor_reduce(out=scol, in_=fTq,
                                    axis=mybir.AxisListType.X,
                                    op=mybir.AluOpType.add)
        misc_ps = psum.tile([1, C_OUT + 1], F32)
        b2s_ps = misc_ps[:, 0:C_OUT]
        scrap = misc_ps[:, C_OUT:C_OUT + 1]
        b2s_sb = work.tile([1, C_OUT], BF16)

        # PE observes the 8 g3 copy semaphores (one wait each) so the main
        # matmuls need only their T-tile wait.
        for k in range(4):
            for hl in range(2):
                nc.tensor.matmul(scrap,
                                 lhsT=g3[hl * H:(hl + 1) * H, 0, 8 * k:8 * k + 1],
                                 rhs=g3[hl * H:(hl + 1) * H, 0, 8 * k:8 * k + 1],
                                 start=True, stop=True)

        # ---- main contraction: acc[a_half, i] += t_p[:,half]^T @ g3[:,p,:]
        # Separate PSUM tiles per a-half: a start=True into a shared tile
        # wipes the other half's accumulation group.
        acc = [psum.tile([2 * H, C_OUT], F32, name=f"acc{i}", tag=f"acc{i}")
               for i in range(2)]
        for p in range(NPAIR):
            for half in range(2):
                nc.tensor.matmul(acc[half],
                                 lhsT=t_tiles[p][:, half * 2 * H:(half + 1) * 2 * H],
                                 rhs=g3[:, p, :],
                                 start=(p == 0), stop=False)

        # ---- bias matmuls: the final ops of each accumulation group (after
        # the mains so the PE never stalls on the scol/b2s chain):
        # b2s[i] = sum_j b2t[j,i] scol[j]; acc[a, (half,i)] += ones^T @ b2s.
        nc.tensor.matmul(b2s_ps, lhsT=scol, rhs=b2t_bf, start=True, stop=True)
        nc.vector.tensor_copy(b2s_sb, b2s_ps)
        for half in range(2):
            nc.tensor.matmul(acc[half], lhsT=ones_r, rhs=b2s_sb,
                             start=False, stop=True)

        # ---- store: out[a, i], a = half*128 + ap.
        out_sb = work.tile([2 * H, 2 * C_OUT], F32)
        nc.vector.tensor_copy(out_sb[:, 0:C_OUT], acc[0])
        nc.vector.tensor_copy(out_sb[:, C_OUT:2 * C_OUT], acc[1])
        src = bass.AP(tensor=out_sb.tensor, offset=out_sb.offset,
                      ap=[[2 * C_OUT, 2 * H], [C_OUT, 2], [1, C_OUT]])
        dst = bass.AP(tensor=outp.tensor, offset=outp.offset,
                      ap=[[C_OUT, 2 * H], [2 * H * C_OUT, 2], [1, C_OUT]])
        nc.sync.dma_start(out=dst, in_=src)

    return nc


def shard_inputs(features, geometry, W1, b1, W2, b2) -> list[dict]:
    import ml_dtypes
    bf16 = ml_dtypes.bfloat16
    f = np.ascontiguousarray(np.asarray(features, np.float32))
    g = np.ascontiguousarray(np.asarray(geometry, np.float32))
    W1 = np.ascontiguousarray(np.asarray(W1, np.float32))
    b1 = np.ascontiguousarray(np.asarray(b1, np.float32))
    W2 = np.ascontiguousarray(np.asarray(W2, np.float32))
    b2 = np.ascontiguousarray(np.asarray(b2, np.float32))

    # m2p2[j, i*64+h] = W2[h, i*C_IN+j]
    m2p2 = W2.reshape(H, C_OUT, C_IN).transpose(2, 1, 0).reshape(C_IN, C_OUT * H)
    b2t = np.ascontiguousarray(b2.reshape(C_OUT, C_IN).T)

    maps = []
    for core in range(8):
        z, q = divmod(core, 4)
        sl = slice(q * BQ, (q + 1) * BQ)
        mp = np.zeros((C_IN, MPW), bf16)
        mp[0:3, 0:256] = g[z].T.astype(bf16)
        mp[0:3, 256:320] = g[z, sl].T.astype(bf16)
        mp[3, 256:320] = 1.0
        mp[0:3, 320:384] = W1.astype(bf16)
        mp[0:3, 384:448] = W1.astype(bf16)
        mp[3, 384:448] = b1.astype(bf16)
        mp[0, 448:576] = 1.0
        mp[:, 576:640] = f[z, sl].T.astype(bf16)
        mp[:, 640:2688] = m2p2.astype(bf16)
        mp[:, 2688:2720] = b2t.astype(bf16)
        maps.append({"mp": mp})
    return maps


def unshard(parts: list[np.ndarray]) -> np.ndarray:
    out = np.empty((Z, N, C_OUT), np.float32)
    for z in range(Z):
        acc = parts[4 * z].astype(np.float32)
        for q in range(1, 4):
            acc = acc + parts[4 * z + q]
        out[z] = acc
    return out


def kernel(**inputs) -> np.ndarray:
    nc = build_nc(debug=False)
    in_maps = shard_inputs(**inputs)
    res = run_bass_kernel_spmd(nc, in_maps, list(range(8)))
    return unshard([r["outp"] for r in res.results])
